# revision 5
# baseline (speedup 1.0000x reference)
"""MoE top-2-of-8 SwiGLU kernel for 8 Trainium2 NeuronCores.

Strategy (expert-parallel, per sharding hint):
  - Router (tiny: N x E x D matmul) + top-2 dispatch computed on host in
    float64; this IS the sharding step — tokens are gathered per expert id.
  - Core e gets expert e's weights (pre-transposed/tiled, fp16) and its
    gathered tokens padded to capacity C = max_e count_e.
  - Device kernel per core: h = silu(x@Wg^T) * (x@Wu^T); y = h@Wd^T.
    All matmuls fp16 operands (1 cycle/row on PE) with fp32 PSUM accum.
  - Host combines: out[n] += w[n,k] * y_row (scatter by the dispatch
    permutation; no atomics needed since top-2 indices are distinct).

Shapes (hardcoded per problem spec): B=2, S=2048, D=1024, H=4096, E=8, K=2.
"""

import numpy as np

import concourse.bass as bass
import concourse.tile as tile
from concourse import bacc, mybir
from concourse.bass_utils import run_bass_kernel_spmd

TOPK = 2
D = 1024
H = 4096
E = 8
NCORES = 8

_compiled_cache = {}
LAST_RUN = None  # BassKernelResults of the most recent SPMD launch


def _chunk_sizes(C, max_chunk=384):
    n = -(-C // max_chunk)
    base = -(-C // n)
    sizes = []
    left = C
    for _ in range(n):
        s = min(base, left)
        sizes.append(s)
        left -= s
    assert sum(sizes) == C and left == 0
    return sizes


def _build_kernel(C, silu_mode="silu"):
    """One SPMD program: given xT [128, D/128, C] fp16 and the expert's
    pre-tiled weights, produce yT [128, D/128, C] fp32.

    silu_mode="sigmoid_mul" avoids the Silu LUT (not implemented in
    CoreSim) by computing sigmoid on ACT and an extra multiply on DVE.
    """
    chunks = _chunk_sizes(C)
    f16 = mybir.dt.float16
    f32 = mybir.dt.float32
    DC = D // 128   # 8 chunks of the contraction dim D
    HT = H // 128   # 32 tiles of the hidden dim H

    nc = bacc.Bacc("TRN2", target_bir_lowering=False, debug=False,
                   num_devices=NCORES)

    xT_d = nc.dram_tensor("xT", [128, DC, C], f16, kind="ExternalInput")
    wg_d = nc.dram_tensor("wg", [128, HT, DC, 128], f16, kind="ExternalInput")
    wu_d = nc.dram_tensor("wu", [128, HT, DC, 128], f16, kind="ExternalInput")
    wd_d = nc.dram_tensor("wd", [128, DC, HT, 128], f16, kind="ExternalInput")
    y_d = nc.dram_tensor("y", [128, DC, C], f32, kind="ExternalOutput")

    with tile.TileContext(nc) as tc:
        with (
            tc.tile_pool(name="xp", bufs=1) as xp,
            tc.tile_pool(name="hp", bufs=1) as hp,
            tc.tile_pool(name="wgp", bufs=3) as wgp,
            tc.tile_pool(name="wup", bufs=3) as wup,
            tc.tile_pool(name="wdp", bufs=2) as wdp,
            tc.tile_pool(name="tmp", bufs=3) as tmpp,
            tc.tile_pool(name="outp", bufs=4) as outp,
            tc.tile_pool(name="pa", bufs=2, space="PSUM") as pap,
            tc.tile_pool(name="pb", bufs=2, space="PSUM") as pbp,
            tc.tile_pool(name="pc", bufs=2, space="PSUM") as pcp,
        ):
            xT = xp.tile([128, DC, C], f16)
            nc.sync.dma_start(xT[:], xT_d[:])
            hT = hp.tile([128, HT, C], f16)

            # Phase 1: hT[:, ht, :] = silu(x @ Wg^T) * (x @ Wu^T), transposed
            for ht in range(HT):
                wg = wgp.tile([128, DC, 128], f16)
                nc.sync.dma_start(wg[:], wg_d[:, ht])
                wu = wup.tile([128, DC, 128], f16)
                nc.sync.dma_start(wu[:], wu_d[:, ht])
                t0 = 0
                for tn in chunks:
                    pA = pap.tile([128, tn], f32)
                    for dc in range(DC):
                        nc.tensor.matmul(pA[:], wg[:, dc], xT[:, dc, t0:t0 + tn],
                                         start=(dc == 0), stop=(dc == DC - 1))
                    pB = pbp.tile([128, tn], f32)
                    for dc in range(DC):
                        nc.tensor.matmul(pB[:], wu[:, dc], xT[:, dc, t0:t0 + tn],
                                         start=(dc == 0), stop=(dc == DC - 1))
                    sl = tmpp.tile([128, tn], f32)
                    if silu_mode == "silu":
                        nc.scalar.activation(sl[:], pA[:],
                                             mybir.ActivationFunctionType.Silu)
                    else:
                        nc.scalar.activation(sl[:], pA[:],
                                             mybir.ActivationFunctionType.Sigmoid)
                        nc.vector.tensor_mul(sl[:], sl[:], pA[:])
                    nc.vector.tensor_mul(hT[:, ht, t0:t0 + tn], sl[:], pB[:])
                    t0 += tn

            # Phase 2: y[:, dt, :] = h @ Wd^T, transposed
            for dt in range(DC):
                wd = wdp.tile([128, HT, 128], f16)
                nc.sync.dma_start(wd[:], wd_d[:, dt])
                t0 = 0
                for tn in chunks:
                    pC = pcp.tile([128, tn], f32)
                    for hc in range(HT):
                        nc.tensor.matmul(pC[:], wd[:, hc], hT[:, hc, t0:t0 + tn],
                                         start=(hc == 0), stop=(hc == HT - 1))
                    ot = outp.tile([128, tn], f32)
                    nc.vector.tensor_copy(ot[:], pC[:])
                    nc.sync.dma_start(y_d[:, dt, t0:t0 + tn], ot[:])
                    t0 += tn

    nc.compile()
    return nc


def _get_kernel(C):
    if C not in _compiled_cache:
        _compiled_cache[C] = _build_kernel(C)
    return _compiled_cache[C]


def _route(xt, Wr):
    """Host router in float64: logits, top-2 (desc, ties by index like
    jax.lax.top_k), renormalized weights."""
    logits64 = xt.astype(np.float64) @ Wr.T.astype(np.float64)
    m = logits64.max(axis=-1, keepdims=True)
    p = np.exp(logits64 - m)
    p /= p.sum(axis=-1, keepdims=True)
    # stable argsort on -p: descending, ties broken by lower index
    idx = np.argsort(-p, axis=-1, kind="stable")[:, :TOPK]
    w = np.take_along_axis(p, idx, axis=-1)
    w /= w.sum(axis=-1, keepdims=True)
    return logits64, idx, w.astype(np.float32)


def kernel(x, Wr, Wg, Wu, Wd):
    B, S, _ = x.shape
    N = B * S
    xt = np.ascontiguousarray(np.asarray(x, dtype=np.float32).reshape(N, D))
    Wr = np.asarray(Wr, dtype=np.float32)

    logits64, idx, w = _route(xt, Wr)

    # dispatch lists per expert (np.where on [N, K] is token-ordered)
    rows_n, rows_k, counts = [], [], []
    for e in range(E):
        rn, rk = np.nonzero(idx == e)
        rows_n.append(rn)
        rows_k.append(rk)
        counts.append(len(rn))
    C = max(max(counts), 1)

    nc = _get_kernel(C)

    xt16 = xt.astype(np.float16)
    in_maps = []
    for e in range(E):
        xe = np.zeros((C, D), dtype=np.float16)
        xe[:counts[e]] = xt16[rows_n[e]]
        # [p, dc, t] = xe[t, dc*128+p]
        xT_t = np.ascontiguousarray(
            xe.T.reshape(D // 128, 128, C).transpose(1, 0, 2))
        wg_t = np.ascontiguousarray(
            np.asarray(Wg[e]).reshape(H // 128, 128, D // 128, 128)
            .transpose(3, 0, 2, 1).astype(np.float16))
        wu_t = np.ascontiguousarray(
            np.asarray(Wu[e]).reshape(H // 128, 128, D // 128, 128)
            .transpose(3, 0, 2, 1).astype(np.float16))
        wd_t = np.ascontiguousarray(
            np.asarray(Wd[e]).reshape(D // 128, 128, H // 128, 128)
            .transpose(3, 0, 2, 1).astype(np.float16))
        in_maps.append({"xT": xT_t, "wg": wg_t, "wu": wu_t, "wd": wd_t})

    global LAST_RUN
    LAST_RUN = run_bass_kernel_spmd(nc, in_maps, list(range(NCORES)))
    results = LAST_RUN.results

    out = np.zeros((N, D), dtype=np.float32)
    for e in range(E):
        cnt = counts[e]
        if cnt == 0:
            continue
        y_t = results[e]["y"]                      # [128, DC, C] f32
        y_tok = y_t.transpose(2, 1, 0).reshape(C, D)[:cnt]
        out[rows_n[e]] += y_tok * w[rows_n[e], rows_k[e]][:, None]

    return out.reshape(B, S, D), logits64.astype(np.float32)


# revision 8
# speedup vs baseline: 1.0009x; 1.0009x over previous
"""MoE top-2-of-8 SwiGLU kernel for 8 Trainium2 NeuronCores.

Strategy (expert-parallel, per sharding hint):
  - Router (tiny: N x E x D matmul) + top-2 dispatch computed on host in
    float64; this IS the sharding step — tokens are gathered per expert id.
  - Core e gets expert e's weights (pre-transposed/tiled, fp16) and its
    gathered tokens padded to capacity C = max_e count_e.
  - Device kernel per core: h = silu(x@Wg^T) * (x@Wu^T); y = h@Wd^T.
    All matmuls fp16 operands (1 cycle/row on PE) with fp32 PSUM accum.
  - Host combines: out[n] += w[n,k] * y_row (scatter by the dispatch
    permutation; no atomics needed since top-2 indices are distinct).

Shapes (hardcoded per problem spec): B=2, S=2048, D=1024, H=4096, E=8, K=2.
"""

import numpy as np

import concourse.bass as bass
import concourse.tile as tile
from concourse import bacc, mybir
from concourse.bass_utils import run_bass_kernel_spmd

TOPK = 2
D = 1024
H = 4096
E = 8
NCORES = 8

_compiled_cache = {}
LAST_RUN = None  # BassKernelResults of the most recent SPMD launch


def _chunk_sizes(C, max_chunk=384):
    n = -(-C // max_chunk)
    base = -(-C // n)
    sizes = []
    left = C
    for _ in range(n):
        s = min(base, left)
        sizes.append(s)
        left -= s
    assert sum(sizes) == C and left == 0
    return sizes


def _build_kernel(C, silu_mode="silu"):
    """One SPMD program: given xT [128, D/128, C] fp16 and the expert's
    pre-tiled weights, produce yT [128, D/128, C] fp32.

    silu_mode="sigmoid_mul" avoids the Silu LUT (not implemented in
    CoreSim) by computing sigmoid on ACT and an extra multiply on DVE.
    """
    chunks = _chunk_sizes(C)
    f16 = mybir.dt.float16
    f32 = mybir.dt.float32
    DC = D // 128   # 8 chunks of the contraction dim D
    HT = H // 128   # 32 tiles of the hidden dim H

    nc = bacc.Bacc("TRN2", target_bir_lowering=False, debug=False,
                   num_devices=NCORES)

    xT_d = nc.dram_tensor("xT", [128, DC, C], f16, kind="ExternalInput")
    wg_d = nc.dram_tensor("wg", [128, HT, DC, 128], f16, kind="ExternalInput")
    wu_d = nc.dram_tensor("wu", [128, HT, DC, 128], f16, kind="ExternalInput")
    wd_d = nc.dram_tensor("wd", [128, DC, HT, 128], f16, kind="ExternalInput")
    y_d = nc.dram_tensor("y", [128, DC, C], f32, kind="ExternalOutput")

    with tile.TileContext(nc) as tc:
        with (
            tc.tile_pool(name="xp", bufs=1) as xp,
            tc.tile_pool(name="hp", bufs=1) as hp,
            tc.tile_pool(name="wgp", bufs=3) as wgp,
            tc.tile_pool(name="wup", bufs=3) as wup,
            tc.tile_pool(name="wdp", bufs=2) as wdp,
            tc.tile_pool(name="tmp", bufs=3) as tmpp,
            tc.tile_pool(name="outp", bufs=4) as outp,
            tc.tile_pool(name="warm", bufs=1) as warmp,
            tc.tile_pool(name="pa", bufs=2, space="PSUM") as pap,
            tc.tile_pool(name="pb", bufs=2, space="PSUM") as pbp,
            tc.tile_pool(name="pc", bufs=2, space="PSUM") as pcp,
        ):
            # PE warm-up: ~4us of dummy matmuls on a zeroed scratch tile so
            # the HAM clock-gate opens (1.2->2.4 GHz) while the input DMAs
            # are still in flight.
            wsrc = warmp.tile([128, 512], f16)
            nc.gpsimd.memset(wsrc[:], 0)
            pw = pap.tile([128, 512], f32)
            for _ in range(10):
                nc.tensor.matmul(pw[:], wsrc[:, :128], wsrc[:], start=True,
                                 stop=True)

            xT = xp.tile([128, DC, C], f16)
            for dc in range(DC):
                nc.sync.dma_start(xT[:, dc], xT_d[:, dc])
            hT = hp.tile([128, HT, C], f16)

            # Phase 1: hT[:, ht, :] = silu(x @ Wg^T) * (x @ Wu^T), transposed
            for ht in range(HT):
                wg = wgp.tile([128, DC, 128], f16)
                nc.sync.dma_start(wg[:], wg_d[:, ht])
                wu = wup.tile([128, DC, 128], f16)
                nc.sync.dma_start(wu[:], wu_d[:, ht])
                t0 = 0
                for tn in chunks:
                    pA = pap.tile([128, tn], f32)
                    for dc in range(DC):
                        nc.tensor.matmul(pA[:, 0:tn], wg[:, dc],
                                         xT[:, dc, t0:t0 + tn],
                                         start=(dc == 0), stop=(dc == DC - 1))
                    pB = pbp.tile([128, tn], f32)
                    for dc in range(DC):
                        nc.tensor.matmul(pB[:, 0:tn], wu[:, dc],
                                         xT[:, dc, t0:t0 + tn],
                                         start=(dc == 0), stop=(dc == DC - 1))
                    sl = tmpp.tile([128, tn], f32)
                    if silu_mode == "silu":
                        nc.scalar.activation(sl[:], pA[:, 0:tn],
                                             mybir.ActivationFunctionType.Silu)
                    else:
                        nc.scalar.activation(sl[:], pA[:, 0:tn],
                                             mybir.ActivationFunctionType.Sigmoid)
                        nc.vector.tensor_mul(sl[:], sl[:], pA[:, 0:tn])
                    nc.vector.tensor_mul(hT[:, ht, t0:t0 + tn], sl[:], pB[:, 0:tn])
                    t0 += tn

            # Phase 2: y[:, dt, :] = h @ Wd^T, transposed
            for dt in range(DC):
                wd = wdp.tile([128, HT, 128], f16)
                nc.sync.dma_start(wd[:], wd_d[:, dt])
                t0 = 0
                for tn in chunks:
                    pC = pcp.tile([128, tn], f32)
                    for hc in range(HT):
                        nc.tensor.matmul(pC[:, 0:tn], wd[:, hc],
                                         hT[:, hc, t0:t0 + tn],
                                         start=(hc == 0), stop=(hc == HT - 1))
                    ot = outp.tile([128, tn], f32)
                    nc.vector.tensor_copy(ot[:], pC[:, 0:tn])
                    nc.sync.dma_start(y_d[:, dt, t0:t0 + tn], ot[:])
                    t0 += tn

    nc.compile()
    return nc


def _get_kernel(C):
    if C not in _compiled_cache:
        _compiled_cache[C] = _build_kernel(C)
    return _compiled_cache[C]


def _route(xt, Wr):
    """Host router in float64: logits, top-2 (desc, ties by index like
    jax.lax.top_k), renormalized weights."""
    logits64 = xt.astype(np.float64) @ Wr.T.astype(np.float64)
    m = logits64.max(axis=-1, keepdims=True)
    p = np.exp(logits64 - m)
    p /= p.sum(axis=-1, keepdims=True)
    # stable argsort on -p: descending, ties broken by lower index
    idx = np.argsort(-p, axis=-1, kind="stable")[:, :TOPK]
    w = np.take_along_axis(p, idx, axis=-1)
    w /= w.sum(axis=-1, keepdims=True)
    return logits64, idx, w.astype(np.float32)


def kernel(x, Wr, Wg, Wu, Wd):
    B, S, _ = x.shape
    N = B * S
    xt = np.ascontiguousarray(np.asarray(x, dtype=np.float32).reshape(N, D))
    Wr = np.asarray(Wr, dtype=np.float32)

    logits64, idx, w = _route(xt, Wr)

    # dispatch lists per expert (np.where on [N, K] is token-ordered)
    rows_n, rows_k, counts = [], [], []
    for e in range(E):
        rn, rk = np.nonzero(idx == e)
        rows_n.append(rn)
        rows_k.append(rk)
        counts.append(len(rn))
    C = max(max(counts), 1)

    nc = _get_kernel(C)

    xt16 = xt.astype(np.float16)
    in_maps = []
    for e in range(E):
        xe = np.zeros((C, D), dtype=np.float16)
        xe[:counts[e]] = xt16[rows_n[e]]
        # [p, dc, t] = xe[t, dc*128+p]
        xT_t = np.ascontiguousarray(
            xe.T.reshape(D // 128, 128, C).transpose(1, 0, 2))
        wg_t = np.ascontiguousarray(
            np.asarray(Wg[e]).reshape(H // 128, 128, D // 128, 128)
            .transpose(3, 0, 2, 1).astype(np.float16))
        wu_t = np.ascontiguousarray(
            np.asarray(Wu[e]).reshape(H // 128, 128, D // 128, 128)
            .transpose(3, 0, 2, 1).astype(np.float16))
        wd_t = np.ascontiguousarray(
            np.asarray(Wd[e]).reshape(D // 128, 128, H // 128, 128)
            .transpose(3, 0, 2, 1).astype(np.float16))
        in_maps.append({"xT": xT_t, "wg": wg_t, "wu": wu_t, "wd": wd_t})

    global LAST_RUN
    LAST_RUN = run_bass_kernel_spmd(nc, in_maps, list(range(NCORES)))
    results = LAST_RUN.results

    out = np.zeros((N, D), dtype=np.float32)
    for e in range(E):
        cnt = counts[e]
        if cnt == 0:
            continue
        y_t = results[e]["y"]                      # [128, DC, C] f32
        y_tok = y_t.transpose(2, 1, 0).reshape(C, D)[:cnt]
        out[rows_n[e]] += y_tok * w[rows_n[e], rows_k[e]][:, None]

    return out.reshape(B, S, D), logits64.astype(np.float32)


# revision 11
# speedup vs baseline: 1.0024x; 1.0014x over previous
"""MoE top-2-of-8 SwiGLU kernel for 8 Trainium2 NeuronCores.

Strategy (expert-parallel, per sharding hint):
  - Router (tiny: N x E x D matmul) + top-2 dispatch computed on host in
    float64; this IS the sharding step — tokens are gathered per expert id.
  - Core e gets expert e's weights (pre-transposed/tiled, fp16) and its
    gathered tokens padded to capacity C = max_e count_e.
  - Device kernel per core: h = silu(x@Wg^T) * (x@Wu^T); y = h@Wd^T.
    All matmuls fp16 operands (1 cycle/row on PE) with fp32 PSUM accum.
  - Host combines: out[n] += w[n,k] * y_row (scatter by the dispatch
    permutation; no atomics needed since top-2 indices are distinct).

Shapes (hardcoded per problem spec): B=2, S=2048, D=1024, H=4096, E=8, K=2.
"""

import numpy as np

import concourse.bass as bass
import concourse.tile as tile
from concourse import bacc, mybir
from concourse.bass_utils import run_bass_kernel_spmd

TOPK = 2
D = 1024
H = 4096
E = 8
NCORES = 8

_compiled_cache = {}
LAST_RUN = None  # BassKernelResults of the most recent SPMD launch


def _chunk_sizes(C, max_chunk=384):
    n = -(-C // max_chunk)
    base = -(-C // n)
    sizes = []
    left = C
    for _ in range(n):
        s = min(base, left)
        sizes.append(s)
        left -= s
    assert sum(sizes) == C and left == 0
    return sizes


def _build_kernel(C, silu_mode="silu"):
    """One SPMD program: given xT [128, D/128, C] fp16 and the expert's
    pre-tiled weights, produce yT [128, D/128, C] fp32.

    silu_mode="sigmoid_mul" avoids the Silu LUT (not implemented in
    CoreSim) by computing sigmoid on ACT and an extra multiply on DVE.
    """
    chunks = _chunk_sizes(C)
    f16 = mybir.dt.float16
    f32 = mybir.dt.float32
    DC = D // 128   # 8 chunks of the contraction dim D
    HT = H // 128   # 32 tiles of the hidden dim H

    nc = bacc.Bacc("TRN2", target_bir_lowering=False, debug=False,
                   num_devices=NCORES)

    xT_d = nc.dram_tensor("xT", [128, DC, C], f16, kind="ExternalInput")
    wg_d = nc.dram_tensor("wg", [128, HT, DC, 128], f16, kind="ExternalInput")
    wu_d = nc.dram_tensor("wu", [128, HT, DC, 128], f16, kind="ExternalInput")
    wd_d = nc.dram_tensor("wd", [128, DC, HT, 128], f16, kind="ExternalInput")
    y_d = nc.dram_tensor("y", [128, DC, C], f32, kind="ExternalOutput")

    with tile.TileContext(nc) as tc:
        with (
            tc.tile_pool(name="xp", bufs=1) as xp,
            tc.tile_pool(name="hp", bufs=1) as hp,
            tc.tile_pool(name="wgp", bufs=3) as wgp,
            tc.tile_pool(name="wup", bufs=3) as wup,
            tc.tile_pool(name="tmp", bufs=3) as tmpp,
            tc.tile_pool(name="outp", bufs=4) as outp,
            tc.tile_pool(name="warm", bufs=1) as warmp,
            tc.tile_pool(name="pa", bufs=2, space="PSUM") as pap,
            tc.tile_pool(name="pb", bufs=2, space="PSUM") as pbp,
            tc.tile_pool(name="pc", bufs=2, space="PSUM") as pcp,
            tc.tile_pool(name="pwarm", bufs=1, space="PSUM") as pwp,
        ):
            # PE warm-up: dummy matmuls on a zeroed scratch tile bridge the
            # initial input-DMA window (~7us) so the HAM clock-gate opens
            # (1.2->2.4 GHz) before the first real matmul and stays open.
            wsrc = warmp.tile([128, 512], f16)
            nc.gpsimd.memset(wsrc[:], 0)
            pw = pwp.tile([128, 512], f32)
            for _ in range(22):
                nc.tensor.matmul(pw[:], wsrc[:, :128], wsrc[:], start=True,
                                 stop=True)

            xT = xp.tile([128, DC, C], f16)
            for dc in range(DC):
                nc.sync.dma_start(xT[:, dc], xT_d[:, dc])
            hT = hp.tile([128, HT, C], f16)

            # Phase 1: hT[:, ht, :] = silu(x @ Wg^T) * (x @ Wu^T), transposed
            # wg and wd share one pool tag: the wd prefetch DMAs then wait
            # for late-phase-1 slot releases instead of stealing HBM
            # bandwidth from the startup-critical wg/wu/xT loads.
            for ht in range(HT):
                wg = wgp.tile([128, DC, 128], f16, tag="w")
                nc.sync.dma_start(wg[:], wg_d[:, ht])
                wu = wup.tile([128, DC, 128], f16)
                nc.sync.dma_start(wu[:], wu_d[:, ht])
                t0 = 0
                for tn in chunks:
                    pA = pap.tile([128, tn], f32)
                    for dc in range(DC):
                        nc.tensor.matmul(pA[:, 0:tn], wg[:, dc],
                                         xT[:, dc, t0:t0 + tn],
                                         start=(dc == 0), stop=(dc == DC - 1))
                    pB = pbp.tile([128, tn], f32)
                    for dc in range(DC):
                        nc.tensor.matmul(pB[:, 0:tn], wu[:, dc],
                                         xT[:, dc, t0:t0 + tn],
                                         start=(dc == 0), stop=(dc == DC - 1))
                    sl = tmpp.tile([128, tn], f32)
                    if silu_mode == "silu":
                        nc.scalar.activation(sl[:], pA[:, 0:tn],
                                             mybir.ActivationFunctionType.Silu)
                    else:
                        nc.scalar.activation(sl[:], pA[:, 0:tn],
                                             mybir.ActivationFunctionType.Sigmoid)
                        nc.vector.tensor_mul(sl[:], sl[:], pA[:, 0:tn])
                    nc.vector.tensor_mul(hT[:, ht, t0:t0 + tn], sl[:], pB[:, 0:tn])
                    t0 += tn

            # Phase 2: y[:, dt, :] = h @ Wd^T, transposed
            for dt in range(DC):
                wd = wgp.tile([128, HT, 128], f16, tag="w")
                nc.sync.dma_start(wd[:], wd_d[:, dt])
                t0 = 0
                for tn in chunks:
                    pC = pcp.tile([128, tn], f32)
                    for hc in range(HT):
                        nc.tensor.matmul(pC[:, 0:tn], wd[:, hc],
                                         hT[:, hc, t0:t0 + tn],
                                         start=(hc == 0), stop=(hc == HT - 1))
                    ot = outp.tile([128, tn], f32)
                    nc.vector.tensor_copy(ot[:], pC[:, 0:tn])
                    nc.sync.dma_start(y_d[:, dt, t0:t0 + tn], ot[:])
                    t0 += tn

    nc.compile()
    return nc


def _get_kernel(C):
    if C not in _compiled_cache:
        _compiled_cache[C] = _build_kernel(C)
    return _compiled_cache[C]


def _route(xt, Wr):
    """Host router in float64: logits, top-2 (desc, ties by index like
    jax.lax.top_k), renormalized weights."""
    logits64 = xt.astype(np.float64) @ Wr.T.astype(np.float64)
    m = logits64.max(axis=-1, keepdims=True)
    p = np.exp(logits64 - m)
    p /= p.sum(axis=-1, keepdims=True)
    # stable argsort on -p: descending, ties broken by lower index
    idx = np.argsort(-p, axis=-1, kind="stable")[:, :TOPK]
    w = np.take_along_axis(p, idx, axis=-1)
    w /= w.sum(axis=-1, keepdims=True)
    return logits64, idx, w.astype(np.float32)


def kernel(x, Wr, Wg, Wu, Wd):
    B, S, _ = x.shape
    N = B * S
    xt = np.ascontiguousarray(np.asarray(x, dtype=np.float32).reshape(N, D))
    Wr = np.asarray(Wr, dtype=np.float32)

    logits64, idx, w = _route(xt, Wr)

    # dispatch lists per expert (np.where on [N, K] is token-ordered)
    rows_n, rows_k, counts = [], [], []
    for e in range(E):
        rn, rk = np.nonzero(idx == e)
        rows_n.append(rn)
        rows_k.append(rk)
        counts.append(len(rn))
    C = max(max(counts), 1)

    nc = _get_kernel(C)

    xt16 = xt.astype(np.float16)
    in_maps = []
    for e in range(E):
        xe = np.zeros((C, D), dtype=np.float16)
        xe[:counts[e]] = xt16[rows_n[e]]
        # [p, dc, t] = xe[t, dc*128+p]
        xT_t = np.ascontiguousarray(
            xe.T.reshape(D // 128, 128, C).transpose(1, 0, 2))
        wg_t = np.ascontiguousarray(
            np.asarray(Wg[e]).reshape(H // 128, 128, D // 128, 128)
            .transpose(3, 0, 2, 1).astype(np.float16))
        wu_t = np.ascontiguousarray(
            np.asarray(Wu[e]).reshape(H // 128, 128, D // 128, 128)
            .transpose(3, 0, 2, 1).astype(np.float16))
        wd_t = np.ascontiguousarray(
            np.asarray(Wd[e]).reshape(D // 128, 128, H // 128, 128)
            .transpose(3, 0, 2, 1).astype(np.float16))
        in_maps.append({"xT": xT_t, "wg": wg_t, "wu": wu_t, "wd": wd_t})

    global LAST_RUN
    LAST_RUN = run_bass_kernel_spmd(nc, in_maps, list(range(NCORES)))
    results = LAST_RUN.results

    out = np.zeros((N, D), dtype=np.float32)
    for e in range(E):
        cnt = counts[e]
        if cnt == 0:
            continue
        y_t = results[e]["y"]                      # [128, DC, C] f32
        y_tok = y_t.transpose(2, 1, 0).reshape(C, D)[:cnt]
        out[rows_n[e]] += y_tok * w[rows_n[e], rows_k[e]][:, None]

    return out.reshape(B, S, D), logits64.astype(np.float32)


# revision 17
# speedup vs baseline: 1.0046x; 1.0022x over previous
"""MoE top-2-of-8 SwiGLU kernel for 8 Trainium2 NeuronCores.

Strategy (expert-parallel, per sharding hint):
  - Router (tiny: N x E x D matmul) + top-2 dispatch computed on host in
    float64; this IS the sharding step — tokens are gathered per expert id.
  - Core e gets expert e's weights (pre-transposed/tiled, fp16) and its
    gathered tokens padded to capacity C = max_e count_e.
  - Device kernel per core: h = silu(x@Wg^T) * (x@Wu^T); y = h@Wd^T.
    All matmuls fp16 operands (1 cycle/row on PE) with fp32 PSUM accum.
  - Host combines: out[n] += w[n,k] * y_row (scatter by the dispatch
    permutation; no atomics needed since top-2 indices are distinct).

Shapes (hardcoded per problem spec): B=2, S=2048, D=1024, H=4096, E=8, K=2.
"""

import numpy as np

import concourse.bass as bass
import concourse.tile as tile
from concourse import bacc, mybir
from concourse.bass_utils import run_bass_kernel_spmd

TOPK = 2
D = 1024
H = 4096
E = 8
NCORES = 8

_compiled_cache = {}
LAST_RUN = None  # BassKernelResults of the most recent SPMD launch


def _chunk_sizes(C, max_chunk=384):
    """Split C into chunks <= max_chunk, all multiples of 8 (except
    possibly the last), sized as evenly as possible."""
    assert C % 8 == 0
    n = -(-C // max_chunk)
    base = -(-C // (8 * n)) * 8
    sizes = []
    left = C
    for _ in range(n):
        s = min(base, left)
        sizes.append(s)
        left -= s
    assert sum(sizes) == C and left == 0
    return [s for s in sizes if s]


def _build_kernel(C, silu_mode="silu"):
    """One SPMD program: given xT [128, D/128, C] fp16 and the expert's
    pre-tiled weights, produce yT [128, D/128, C] fp32.

    silu_mode="sigmoid_mul" avoids the Silu LUT (not implemented in
    CoreSim) by computing sigmoid on ACT and an extra multiply on DVE.
    """
    chunks = _chunk_sizes(C)
    f16 = mybir.dt.float16
    f32 = mybir.dt.float32
    DC = D // 128   # 8 chunks of the contraction dim D
    HT = H // 128   # 32 tiles of the hidden dim H

    nc = bacc.Bacc("TRN2", target_bir_lowering=False, debug=False,
                   num_devices=NCORES)

    xT_d = nc.dram_tensor("xT", [128, DC, C], f16, kind="ExternalInput")
    wg_d = nc.dram_tensor("wg", [128, HT, DC, 128], f16, kind="ExternalInput")
    wu_d = nc.dram_tensor("wu", [128, HT, DC, 128], f16, kind="ExternalInput")
    wd_d = nc.dram_tensor("wd", [128, DC, HT, 128], f16, kind="ExternalInput")
    y_d = nc.dram_tensor("y", [128, DC, C], f32, kind="ExternalOutput")

    with tile.TileContext(nc) as tc:
        with (
            tc.tile_pool(name="xp", bufs=1) as xp,
            tc.tile_pool(name="hp", bufs=1) as hp,
            tc.tile_pool(name="wgp", bufs=3) as wgp,
            tc.tile_pool(name="wup", bufs=3) as wup,
            tc.tile_pool(name="tmp", bufs=3) as tmpp,
            tc.tile_pool(name="outp", bufs=4) as outp,
            tc.tile_pool(name="warm", bufs=1) as warmp,
            tc.tile_pool(name="pa", bufs=2, space="PSUM") as pap,
            tc.tile_pool(name="pb", bufs=2, space="PSUM") as pbp,
            tc.tile_pool(name="pc", bufs=2, space="PSUM") as pcp,
            tc.tile_pool(name="pwarm", bufs=1, space="PSUM") as pwp,
        ):
            # PE warm-up: dummy matmuls on a zeroed scratch tile bridge the
            # initial input-DMA window so the HAM clock-gate opens
            # (1.2->2.4 GHz) before the first real matmul.
            wsrc = warmp.tile([128, 512], f16)
            nc.gpsimd.memset(wsrc[:], 0)
            pw = pwp.tile([128, 512], f32)
            for _ in range(9):
                nc.tensor.matmul(pw[:], wsrc[:, :128], wsrc[:], start=True,
                                 stop=True)

            # Startup-critical DMAs: first weight tiles ahead of x in the
            # sync queue; x split per-dc across sync+gpsimd so triggers
            # issue in parallel (SWDGE trigger serialization ~1us each).
            wg0 = wgp.tile([128, DC, 128], f16, tag="w")
            nc.sync.dma_start(wg0[:], wg_d[:, 0])
            wu0 = wup.tile([128, DC, 128], f16)
            nc.sync.dma_start(wu0[:], wu_d[:, 0])
            xT = xp.tile([128, DC, C], f16)
            for dc in range(DC):
                eng = nc.sync if dc % 2 == 0 else nc.gpsimd
                eng.dma_start(xT[:, dc], xT_d[:, dc])
            hT = hp.tile([128, HT, C], f16)

            # Phase 1: hT[:, ht, :] = silu(x @ Wg^T) * (x @ Wu^T), transposed
            # wg and wd share one pool tag: the wd prefetch DMAs then wait
            # for late-phase-1 slot releases instead of stealing HBM
            # bandwidth from the startup-critical wg/wu/xT loads.
            for ht in range(HT):
                if ht == 0:
                    wg, wu = wg0, wu0
                else:
                    wg = wgp.tile([128, DC, 128], f16, tag="w")
                    nc.sync.dma_start(wg[:], wg_d[:, ht])
                    wu = wup.tile([128, DC, 128], f16)
                    nc.sync.dma_start(wu[:], wu_d[:, ht])
                sl = tmpp.tile([128, C], f32)
                t0 = 0
                for tn in chunks:
                    pA = pap.tile([128, tn], f32)
                    for dc in range(DC):
                        nc.tensor.matmul(pA[:, 0:tn], wg[:, dc],
                                         xT[:, dc, t0:t0 + tn],
                                         start=(dc == 0), stop=(dc == DC - 1))
                    pB = pbp.tile([128, tn], f32)
                    for dc in range(DC):
                        nc.tensor.matmul(pB[:, 0:tn], wu[:, dc],
                                         xT[:, dc, t0:t0 + tn],
                                         start=(dc == 0), stop=(dc == DC - 1))
                    slc = sl[:, t0:t0 + tn]
                    if silu_mode == "silu":
                        nc.scalar.activation(slc, pA[:, 0:tn],
                                             mybir.ActivationFunctionType.Silu)
                    else:
                        nc.scalar.activation(slc, pA[:, 0:tn],
                                             mybir.ActivationFunctionType.Sigmoid)
                        nc.vector.tensor_mul(slc, slc, pA[:, 0:tn])
                    nc.vector.tensor_mul(hT[:, ht, t0:t0 + tn], slc, pB[:, 0:tn])
                    t0 += tn

            # Phase 2: y[:, dt, :] = h @ Wd^T, transposed
            for dt in range(DC):
                wd = wgp.tile([128, HT, 128], f16, tag="w")
                nc.sync.dma_start(wd[:], wd_d[:, dt])
                ot = outp.tile([128, C], f32)
                t0 = 0
                for tn in chunks:
                    pC = pcp.tile([128, tn], f32)
                    for hc in range(HT):
                        nc.tensor.matmul(pC[:, 0:tn], wd[:, hc],
                                         hT[:, hc, t0:t0 + tn],
                                         start=(hc == 0), stop=(hc == HT - 1))
                    nc.vector.tensor_copy(ot[:, t0:t0 + tn], pC[:, 0:tn])
                    nc.sync.dma_start(y_d[:, dt, t0:t0 + tn], ot[:, t0:t0 + tn])
                    t0 += tn

    nc.compile()
    return nc


def _get_kernel(C):
    if C not in _compiled_cache:
        _compiled_cache[C] = _build_kernel(C)
    return _compiled_cache[C]


def _route(xt, Wr):
    """Host router in float64: logits, top-2 (desc, ties by index like
    jax.lax.top_k), renormalized weights."""
    logits64 = xt.astype(np.float64) @ Wr.T.astype(np.float64)
    m = logits64.max(axis=-1, keepdims=True)
    p = np.exp(logits64 - m)
    p /= p.sum(axis=-1, keepdims=True)
    # stable argsort on -p: descending, ties broken by lower index
    idx = np.argsort(-p, axis=-1, kind="stable")[:, :TOPK]
    w = np.take_along_axis(p, idx, axis=-1)
    w /= w.sum(axis=-1, keepdims=True)
    return logits64, idx, w.astype(np.float32)


def kernel(x, Wr, Wg, Wu, Wd):
    B, S, _ = x.shape
    N = B * S
    xt = np.ascontiguousarray(np.asarray(x, dtype=np.float32).reshape(N, D))
    Wr = np.asarray(Wr, dtype=np.float32)

    logits64, idx, w = _route(xt, Wr)

    # dispatch lists per expert (np.where on [N, K] is token-ordered)
    rows_n, rows_k, counts = [], [], []
    for e in range(E):
        rn, rk = np.nonzero(idx == e)
        rows_n.append(rn)
        rows_k.append(rk)
        counts.append(len(rn))
    C = -(-max(max(counts), 8) // 8) * 8  # capacity, multiple of 8

    nc = _get_kernel(C)

    xt16 = xt.astype(np.float16)
    in_maps = []
    for e in range(E):
        xe = np.zeros((C, D), dtype=np.float16)
        xe[:counts[e]] = xt16[rows_n[e]]
        # [p, dc, t] = xe[t, dc*128+p]
        xT_t = np.ascontiguousarray(
            xe.T.reshape(D // 128, 128, C).transpose(1, 0, 2))
        wg_t = np.ascontiguousarray(
            np.asarray(Wg[e]).reshape(H // 128, 128, D // 128, 128)
            .transpose(3, 0, 2, 1).astype(np.float16))
        wu_t = np.ascontiguousarray(
            np.asarray(Wu[e]).reshape(H // 128, 128, D // 128, 128)
            .transpose(3, 0, 2, 1).astype(np.float16))
        wd_t = np.ascontiguousarray(
            np.asarray(Wd[e]).reshape(D // 128, 128, H // 128, 128)
            .transpose(3, 0, 2, 1).astype(np.float16))
        in_maps.append({"xT": xT_t, "wg": wg_t, "wu": wu_t, "wd": wd_t})

    global LAST_RUN
    LAST_RUN = run_bass_kernel_spmd(nc, in_maps, list(range(NCORES)))
    results = LAST_RUN.results

    out = np.zeros((N, D), dtype=np.float32)
    for e in range(E):
        cnt = counts[e]
        if cnt == 0:
            continue
        y_t = results[e]["y"]                      # [128, DC, C] f32
        y_tok = y_t.transpose(2, 1, 0).reshape(C, D)[:cnt]
        out[rows_n[e]] += y_tok * w[rows_n[e], rows_k[e]][:, None]

    return out.reshape(B, S, D), logits64.astype(np.float32)
